# revision 2
# baseline (speedup 1.0000x reference)
"""Trainium2 Bass kernel for nn_ArbitraryRNN (4-layer masked Elman RNN).

kernel(**inputs) takes the FULL inputs (x [2048,64,256] plus 256x256
weights/biases/masks), runs a distributed Bass kernel SPMD on 8
NeuronCores, and returns the full [64,256] output (last timestep of
layer2 + skip recurrence sums).

Strategy: data-parallel over batch (8 cores x B=8; weights replicated —
the sequence dim cannot be sharded due to the recurrence). Each core
runs all four recurrences ("lanes": L0, L1, L2, Ls=skip) chunk-pipelined
as a wavefront: in round r, L0 processes chunk r, L1/Ls chunk r-1, L2
chunk r-2, so the four serial tanh chains interleave on the engines.

Per chunk (C=64 steps) the input transform xg = wihT.T @ h_prev plus
bias (a rank-1 ones-matmul) is bulk-matmul'd straight into PSUM; one
PSUM bank per output half, so the start=True matmul's bank-wide
has_written clear exactly covers its own region. Per-step recurrent
matmuls accumulate on top (start=False) and ScalarE tanh reads PSUM and
writes the hidden state H-major into SBUF, where it feeds both the next
step's matmul rhs and the consumer lane's bulk rhs directly (no
transposes anywhere: contraction always has H on partitions).

Lanes {L0,L2} and {L1,Ls} share chunk parity, so each pair's per-step
tanh is ONE merged ScalarE activation over adjacent PSUM regions —
halving the dominant ACT fixed overhead. Weights/x/h are bf16 (PSUM
accumulation stays fp32; measured rel err ~4e-3), which halves the
per-step LDWEIGHTS cost that dominates the tensor engine.
"""

import numpy as np

T, B_TOTAL, H = 2048, 64, 256
N_CORES = 8
B = B_TOTAL // N_CORES  # 8
C = 64                  # C*B*4B = one 2KB PSUM bank per m-half
KH = MH = 2

LANES = [0, 1, 2, 3]  # L0, L1, L2, Ls
LAG = {0: 0, 1: 1, 2: 2, 3: 1}
PROD = {1: 0, 2: 1, 3: 0}
CONS = {0: [1, 3], 1: [2], 2: [], 3: []}
POS = {0: 0, 2: 1, 1: 2, 3: 3}   # position in merged psum/h tensors
PAIR = {0: 0, 2: 0, 1: 1, 3: 1}
PAIR_LANES = {0: [0, 2], 1: [1, 3]}
PAIR_LAG = {0: 0, 1: 1}


def _build(dt):
    import concourse.bass as bass
    import concourse.mybir as mybir

    F32 = mybir.dt.float32
    R = T // C
    TOTAL_ROUNDS = R + 2
    assert C * B == 512

    nc = bass.Bass()

    xT = nc.declare_dram_parameter("xT", [2, 128, T, B], dt, isOutput=False)
    whhT = nc.declare_dram_parameter("whhT", [4, 256, 256], dt, isOutput=False)
    wihT = nc.declare_dram_parameter("wihT", [4, 256, 256], dt, isOutput=False)
    biasP = nc.declare_dram_parameter("bias", [1, 4, 256], F32, isOutput=False)
    outP = nc.declare_dram_parameter("out", [2, 128, B], F32, isOutput=True)

    cms = []

    def ent(cm):
        cms.append(cm)
        return cm.__enter__()

    whh_sb = ent(nc.sbuf_tensor("whh_sb", [128, 4, KH, MH, 128], dt))
    wih_sb = ent(nc.sbuf_tensor("wih_sb", [128, 4, KH, MH, 128], dt))
    bias_sb = ent(nc.sbuf_tensor("bias_sb", [128, 4, MH, 128], F32))
    ones_sb = ent(nc.sbuf_tensor("ones_sb", [128, C * B], F32))
    hzero = ent(nc.sbuf_tensor("hzero", [128, KH, B], dt))
    x_sb = ent(nc.sbuf_tensor("x_sb", [128, 2, KH, C, B], dt))
    h_all = ent(nc.sbuf_tensor("h_all", [128, 4, 2, KH, C, B], dt))
    out_sb = ent(nc.sbuf_tensor("out_sb", [128, MH, B], F32))

    ps_all = ent(nc.psum_tensor("ps_all", [128, 4, MH, C, B], F32))

    s_hp = [ent(nc.semaphore(f"s_hp{p}")) for p in range(2)]
    s_mm = [ent(nc.semaphore(f"s_mm{l}")) for l in LANES]
    s_blk = [ent(nc.semaphore(f"s_blk{l}")) for l in LANES]
    s_xdma = ent(nc.semaphore("s_xdma"))
    s_init = ent(nc.semaphore("s_init"))
    s_fin = ent(nc.semaphore("s_fin"))
    s_out = ent(nc.semaphore("s_out"))

    block = ent(nc.Block())

    def chunk_of(lane, r):
        return r - LAG[lane]

    def active(lane, r):
        return 0 <= chunk_of(lane, r) < R

    def hp_thresh(lane, n):
        return n + (LAG[lane] - PAIR_LAG[PAIR[lane]]) * C

    n_init = {"n": 0}

    @block.sync
    def _(sync):
        for l in range(4):
            for k in range(KH):
                for m in range(MH):
                    sync.dma_start(
                        out=whh_sb[:, l, k, m, :],
                        in_=whhT.ap()[l, k * 128 : (k + 1) * 128, m * 128 : (m + 1) * 128],
                    ).then_inc(s_init, 16)
                    sync.dma_start(
                        out=wih_sb[:, l, k, m, :],
                        in_=wihT.ap()[l, k * 128 : (k + 1) * 128, m * 128 : (m + 1) * 128],
                    ).then_inc(s_init, 16)
                    n_init["n"] += 2
            sync.dma_start(
                out=bias_sb[:1, l, :, :],
                in_=biasP.ap()[0:1, l, :].rearrange("o (mh ml) -> o mh ml", ml=128),
            ).then_inc(s_init, 16)
            n_init["n"] += 1
        for c in range(R):
            if c >= 2:
                sync.wait_ge(s_blk[0], c - 1)
            sync.dma_start(
                out=x_sb[:, c % 2, :, :, :],
                in_=xT.ap()[:, :, c * C : (c + 1) * C, :].rearrange(
                    "kh kl t b -> kl kh t b"
                ),
            ).then_inc(s_xdma, 16)
        sync.wait_ge(s_fin, 1)
        sync.dma_start(
            out=outP.ap().rearrange("mh ml b -> ml mh b"), in_=out_sb[:, :, :]
        ).then_inc(s_out, 16)
        sync.wait_ge(s_out, 16)

    @block.gpsimd
    def _(gpsimd):
        gpsimd.memset(ones_sb[:1, :], 1.0).then_inc(s_init, 1)
        gpsimd.memset(hzero[:, :, :], 0.0).then_inc(s_init, 1)

    INIT_THRESH = n_init["n"] * 16 + 2

    @block.tensor
    def _(pe):
        pe.wait_ge(s_init, INIT_THRESH)

        def emit_bulk(lane, c):
            p = POS[lane]
            last = None
            for m in range(MH):
                outap = ps_all[:, p, m, :, :]
                for k in range(KH):
                    if lane == 0:
                        rhs = x_sb[:, c % 2, k, :, :]
                    else:
                        rhs = h_all[:, POS[PROD[lane]], c % 2, k, :, :]
                    pe.matmul(
                        outap,
                        wih_sb[:, lane, k, m, :],
                        rhs,
                        start=(k == 0),
                        stop=False,
                        skip_group_check=True,
                    )
                last = pe.matmul(
                    outap,
                    bias_sb[:1, lane, m, :],
                    ones_sb[:1, :],
                    start=False,
                    stop=False,
                    skip_group_check=True,
                )
            last.then_inc(s_blk[lane], 1)

        def emit_rec_step(lane, c, t, need_wait):
            n = c * C + t
            if need_wait and n >= 1:
                pe.wait_ge(s_hp[PAIR[lane]], hp_thresh(lane, n))
            p = POS[lane]
            ins = None
            for m in range(MH):
                outap = ps_all[:, p, m, t, :]
                for k in range(KH):
                    if t == 0:
                        if c == 0:
                            rhs = hzero[:, k, :]
                        else:
                            rhs = h_all[:, p, (c - 1) % 2, k, C - 1, :]
                    else:
                        rhs = h_all[:, p, c % 2, k, t - 1, :]
                    ins = pe.matmul(
                        outap,
                        whh_sb[:, lane, k, m, :],
                        rhs,
                        start=False,
                        stop=(m == MH - 1 and k == KH - 1),
                        skip_group_check=True,
                    )
            ins.then_inc(s_mm[lane], 1)

        for r in range(TOTAL_ROUNDS):
            lanes_now = [l for l in LANES if active(l, r)]
            for lane in lanes_now:
                c = chunk_of(lane, r)
                if c >= 1:
                    pe.wait_ge(s_hp[PAIR[lane]], hp_thresh(lane, c * C))
                if lane == 0:
                    pe.wait_ge(s_xdma, 16 * (c + 1))
                else:
                    pl = PROD[lane]
                    pe.wait_ge(s_hp[PAIR[pl]], hp_thresh(pl, (c + 1) * C))
                emit_bulk(lane, c)
            for t in range(C):
                for pid in range(2):
                    plist = [l for l in PAIR_LANES[pid] if l in lanes_now]
                    first = True
                    for lane in plist:
                        emit_rec_step(lane, chunk_of(lane, r), t, need_wait=first)
                        first = False

    @block.scalar
    def _(scalar):
        import concourse.mybir as mybir

        for r in range(TOTAL_ROUNDS):
            lanes_now = [l for l in LANES if active(l, r)]
            for t in range(C):
                for pid in range(2):
                    plist = [l for l in PAIR_LANES[pid] if l in lanes_now]
                    if not plist:
                        continue
                    for lane in plist:
                        c = chunk_of(lane, r)
                        if t == 0 and c >= 2:
                            for cons in CONS[lane]:
                                scalar.wait_ge(s_blk[cons], c - 1)
                        scalar.wait_ge(s_mm[lane], c * C + t + 1)
                    c0 = chunk_of(plist[0], r)
                    buf = c0 % 2  # pair lanes share chunk parity
                    p0 = POS[plist[0]]
                    npos = len(plist)
                    scalar.activation(
                        h_all[:, p0 : p0 + npos, buf, :, t, :],
                        ps_all[:, p0 : p0 + npos, :, t, :],
                        mybir.ActivationFunctionType.Tanh,
                    ).then_inc(s_hp[pid], 1)

    @block.vector
    def _(vector):
        R_ = T // C
        vector.wait_ge(s_hp[0], hp_thresh(2, T))
        vector.wait_ge(s_hp[1], hp_thresh(3, T))
        h2 = h_all[:, POS[2], (R_ - 1) % 2, :, C - 1, :]
        hs = h_all[:, POS[3], (R_ - 1) % 2, :, C - 1, :]
        vector.tensor_add(out_sb[:, :, :], h2, hs).then_inc(s_fin, 1)

    for cm in reversed(cms):
        cm.__exit__(None, None, None)
    return nc


def _prep_inputs(inputs, dt_np):
    x = np.asarray(inputs["x"], dtype=np.float32)
    names = ["0", "1", "2", "s"]
    whhT = np.stack([np.asarray(inputs[f"w_hh{n}"], dtype=np.float32).T for n in names])
    masks = [
        None,
        np.asarray(inputs["mask1"]),
        np.asarray(inputs["mask2"]),
        np.asarray(inputs["mask_skip"]),
    ]
    wihT_l = []
    for li, n in enumerate(names):
        w = np.asarray(inputs[f"w_ih{n}"], dtype=np.float32)
        if masks[li] is not None:
            w = w * masks[li].astype(np.float32)
        wihT_l.append(w.T)
    wihT = np.stack(wihT_l)
    bias = np.stack(
        [
            np.asarray(inputs[f"b_ih{n}"], dtype=np.float32)
            + np.asarray(inputs[f"b_hh{n}"], dtype=np.float32)
            for n in names
        ]
    )[None]

    whhT = whhT.astype(dt_np)
    wihT = wihT.astype(dt_np)
    bias = bias.astype(np.float32)

    in_maps = []
    for g in range(N_CORES):
        xg = x[:, g * B : (g + 1) * B, :]
        xTg = np.ascontiguousarray(xg.transpose(2, 0, 1).reshape(2, 128, T, B)).astype(
            dt_np
        )
        in_maps.append({"xT": xTg, "whhT": whhT, "wihT": wihT, "bias": bias})
    return in_maps


_CACHE = {}


def kernel(**inputs) -> np.ndarray:
    import ml_dtypes
    import concourse.mybir as mybir
    from concourse.bass_utils import run_bass_kernel_spmd

    dt = mybir.dt.bfloat16
    dt_np = ml_dtypes.bfloat16

    if "nc" not in _CACHE:
        _CACHE["nc"] = _build(dt)
    nc = _CACHE["nc"]

    in_maps = _prep_inputs(inputs, dt_np)
    res = run_bass_kernel_spmd(nc, in_maps, core_ids=list(range(N_CORES)))

    outs = []
    for g in range(N_CORES):
        o = np.asarray(res.results[g]["out"], dtype=np.float32)  # [2, 128, B]
        outs.append(o.reshape(H, B).T)
    return np.concatenate(outs, axis=0).astype(np.float32)
